# revision 6
# baseline (speedup 1.0000x reference)
"""GraphSAGE conv layer (PyG SAGEConv, aggr='mean') on 8 Trainium2 NeuronCores.

    out = relu(mean_j(x[src_j]) @ W_l + b_l + x @ W_r)

Sharding: destination nodes are assigned to the 8 cores by a global
degree-balanced bin packing (196 bins x 32 nodes per core); the small 128x128
weights are replicated.

The host does all per-edge indexing; the device is a streaming pipeline with
no gathers:

  - Nodes are dealt into 1568 bins so every bin receives ~equal edge counts;
    bins are then matched across cores by load so one shared per-column
    schedule (bin boundaries, start/stop flags, single NEFF) fits all 8 cores
    with <2% padded edge slots.
  - The per-edge source features stream from HBM as fp8 (e4m3) and are
    converted on-chip to bf16 (exact: e4m3 c bf16) by DVE/GpSimd, halving
    the dominant HBM stream while keeping bf16 stationary operands - fp8
    stationaries stall the PE weight-load pipeline (~122ns/matmul vs ~27ns).
  - PE: per column, one matmul msgs^T @ onehot (bf16 x fp8) accumulates the
    feature-major per-node segment sum into a PSUM group tile.
  - DVE: multiplies the PSUM sums by 1/deg while casting to bf16; the
    reciprocal table streams as a single 12.5KB row and is replicated
    across partitions on-chip (GpSimd partition_broadcast).
  - PE: weight-stationary bf16 matmuls add W_l^T @ meanT + W_r^T @ xT.
  - ACT: fused bias + ReLU; result stored feature-major as bf16 and the host
    unshuffles/transposes while assembling the full output.

All loads are issued up front (deep prefetch): the fp8 message + one-hot
streams on the sync DMA queue, x^T/constants on the gpsimd queue, stores on
the scalar queue.
"""

import math

import numpy as np

N_CORES = 8
D = 128
P = 128
BIN = 32            # nodes per psum bin (one-hot width)
GROUP_BINS = 16     # bins per psum group -> 512 nodes
NBINS = 196         # bins per core


# ---------------------------------------------------------------------------
# Host-side sharding / stream prep
# ---------------------------------------------------------------------------

def _prep(x, src, dst, n_cores):
    import ml_dtypes

    fp8 = ml_dtypes.float8_e4m3fn
    bf16 = ml_dtypes.bfloat16

    n, d = x.shape
    assert d == D
    nbins_tot = n_cores * NBINS               # 1568
    nrank = NBINS * BIN                       # 6272
    n_groups = math.ceil(NBINS / GROUP_BINS)  # 13
    assert n <= nbins_tot * BIN

    deg = np.bincount(dst, minlength=n).astype(np.int64)
    recip = np.zeros(n, dtype=np.float32)
    nz = deg > 0
    recip[nz] = 1.0 / deg[nz]

    # --- balanced bin packing: snake-deal nodes (sorted by degree) into bins
    order = np.argsort(-deg, kind="stable")
    bin_of_node = np.empty(n, dtype=np.int64)
    slot_of_node = np.empty(n, dtype=np.int64)
    for r in range(math.ceil(n / nbins_tot)):
        chunk = order[r * nbins_tot:(r + 1) * nbins_tot]
        k = len(chunk)
        b = np.arange(k) if r % 2 == 0 else nbins_tot - 1 - np.arange(k)
        bin_of_node[chunk] = b
        slot_of_node[chunk] = r

    # bin load = number of edges whose dst falls in the bin
    load = np.bincount(bin_of_node[dst], minlength=nbins_tot)

    # --- match bins across cores by load: rank i takes the 8 bins of ranks
    # [8i, 8i+8); the shared schedule then pads each core's bin to the max
    # load in its rank group (within a couple of edges of its own count).
    bsort = np.argsort(-load, kind="stable")
    core_of_bin = np.empty(nbins_tot, dtype=np.int64)
    index_of_bin = np.empty(nbins_tot, dtype=np.int64)
    for i in range(NBINS):
        grp = bsort[i * n_cores:(i + 1) * n_cores]
        cores = np.arange(n_cores) if i % 2 == 0 else np.arange(n_cores - 1, -1, -1)
        core_of_bin[grp] = cores
        index_of_bin[grp] = i
    t_need = np.maximum(load[bsort].reshape(NBINS, n_cores)[:, 0], 1)

    C_b = (t_need + P - 1) // P
    F_b = C_b - 1
    # tails quantized to 32 rows: PE matmul operands must sit at partition
    # offsets 0/32/64/96 (and sizes >64 only at 0), so slots are 32-granular.
    t_tail = 32 * ((np.maximum(t_need - F_b * P, 1) + 31) // 32)

    # --- shared column schedule: full 128-row columns per bin plus tails
    # first-fit-decreasing packed into shared 128-row blocks per psum group.
    full_base = np.zeros(NBINS, dtype=np.int64)
    block_col = np.zeros(NBINS, dtype=np.int64)
    block_off = np.zeros(NBINS, dtype=np.int64)
    groups = []          # (c0, c1, wg)
    sched = []           # per group: list of (c, p0, t, win_lo, start, stop)
    ncol = 0
    for g in range(n_groups):
        b0, b1 = g * GROUP_BINS, min((g + 1) * GROUP_BINS, NBINS)
        c0 = ncol
        for b in range(b0, b1):
            full_base[b] = ncol
            ncol += int(F_b[b])
        order2 = sorted(range(b0, b1), key=lambda b: -t_tail[b])
        blocks = []      # (col, remaining rows)
        for b in order2:
            t = int(t_tail[b])
            placed = False
            for bi, (bc, rem) in enumerate(blocks):
                if rem >= t:
                    block_col[b] = bc
                    block_off[b] = P - rem
                    blocks[bi] = (bc, rem - t)
                    placed = True
                    break
            if not placed:
                block_col[b] = ncol
                block_off[b] = 0
                blocks.append((ncol, P - t))
                ncol += 1
        gsched = []
        for b in range(b0, b1):
            for k in range(int(F_b[b])):
                gsched.append((int(full_base[b] + k), 0, P, (b - b0) * BIN,
                               k == 0, False))
            gsched.append((int(block_col[b]), int(block_off[b]), int(t_tail[b]),
                           (b - b0) * BIN, int(F_b[b]) == 0, True))
        groups.append((c0, ncol, (b1 - b0) * BIN))
        sched.append(gsched)
    n_cols = ncol
    maxc = max(c1 - c0 for c0, c1, _ in groups)

    # --- per-core streams
    core_of_node = core_of_bin[bin_of_node]
    pos_of_node = index_of_bin[bin_of_node] * BIN + slot_of_node
    x_bf = x.astype(bf16)
    x_f8 = x.astype(fp8)
    edge_core = core_of_node[dst]

    in_parts = []
    node_pos = []        # (node ids, local positions) per core, for unshuffle
    for m in range(n_cores):
        sel = edge_core == m
        s = src[sel]
        dn = dst[sel]
        bi = index_of_bin[bin_of_node[dn]]
        order3 = np.argsort(bi, kind="stable")
        s, dn, bi = s[order3], dn[order3], bi[order3]
        cnt = np.bincount(bi, minlength=NBINS)
        bin_start = np.concatenate([[0], np.cumsum(cnt)])
        j = np.arange(len(s)) - bin_start[bi]
        fullslots = F_b[bi] * P
        is_full = j < fullslots
        col = np.where(is_full, full_base[bi] + (j >> 7), block_col[bi])
        p = np.where(is_full, j & 127, block_off[bi] + (j - fullslots))
        lin = col * P + p

        msgs = np.zeros((n_cols * P, D), dtype=fp8)
        msgs[lin] = x_f8[s]
        msgs = msgs.reshape(n_cols, P, D).transpose(1, 0, 2)
        msgs = np.ascontiguousarray(msgs.reshape(P, n_cols * D))

        oh = np.zeros((n_cols * P, BIN), dtype=np.float32)
        oh[lin, slot_of_node[dn]] = 1.0
        oh = oh.reshape(n_cols, P, BIN).transpose(1, 0, 2)
        oh = np.ascontiguousarray(oh.reshape(P, n_cols * BIN)).astype(fp8)

        nodes_m = np.nonzero(core_of_node == m)[0]
        pos_m = pos_of_node[nodes_m]
        xt = np.zeros((P, nrank), dtype=bf16)
        xt[:, pos_m] = x_bf[nodes_m].T
        rc = np.zeros((1, nrank), dtype=bf16)
        rc[0, pos_m] = recip[nodes_m].astype(bf16)

        in_parts.append({
            "msgs": msgs,
            "oh": oh,
            "xt": np.ascontiguousarray(xt),
            "recip": np.ascontiguousarray(rc),
        })
        node_pos.append((nodes_m, pos_m))

    meta = {
        "n": n, "nrank": nrank, "n_cols": n_cols,
        "n_groups": n_groups, "groups": groups, "maxc": maxc,
        "sched": sched, "node_pos": node_pos,
    }
    return meta, in_parts


# ---------------------------------------------------------------------------
# Device kernel builder
# ---------------------------------------------------------------------------

def _build(meta):
    from contextlib import ExitStack

    import concourse.bass as bass  # noqa: F401
    import concourse.mybir as mybir
    import concourse.tile as tile
    from concourse import bacc

    f32 = mybir.dt.float32
    bf16 = mybir.dt.bfloat16
    fp8 = mybir.dt.float8e4
    nrank = meta["nrank"]
    n_cols = meta["n_cols"]
    groups = meta["groups"]
    maxc = meta["maxc"]
    sched = meta["sched"]
    n_groups = meta["n_groups"]

    nc = bacc.Bacc("TRN2", target_bir_lowering=False)
    msgs_d = nc.dram_tensor("msgs", [P, n_cols * D], fp8, kind="ExternalInput")
    oh_d = nc.dram_tensor("oh", [P, n_cols * BIN], fp8, kind="ExternalInput")
    xt_d = nc.dram_tensor("xt", [P, nrank], bf16, kind="ExternalInput")
    rc_d = nc.dram_tensor("recip", [1, nrank], bf16, kind="ExternalInput")
    wl_d = nc.dram_tensor("wl", [D, D], bf16, kind="ExternalInput")
    wr_d = nc.dram_tensor("wr", [D, D], bf16, kind="ExternalInput")
    b_d = nc.dram_tensor("bias", [D, 1], f32, kind="ExternalInput")
    out_d = nc.dram_tensor("outT", [P, nrank], bf16, kind="ExternalOutput")

    goff = [0]
    for _, _, wg in groups:
        goff.append(goff[-1] + wg)

    with ExitStack() as ctx:
        tc = ctx.enter_context(tile.TileContext(nc))
        const = ctx.enter_context(tc.tile_pool(name="const", bufs=1))
        msg8_pool = ctx.enter_context(tc.tile_pool(name="msg8", bufs=6))
        msg16_pool = ctx.enter_context(tc.tile_pool(name="msg16", bufs=5))
        oh_pool = ctx.enter_context(tc.tile_pool(name="ohp", bufs=n_groups))
        xt_pool = ctx.enter_context(tc.tile_pool(name="xtp", bufs=n_groups))
        mean_pool = ctx.enter_context(tc.tile_pool(name="mean", bufs=2))
        out_pool = ctx.enter_context(tc.tile_pool(name="outp", bufs=2))
        mt_psum = ctx.enter_context(tc.tile_pool(name="mtps", bufs=4, space="PSUM"))
        z_psum = ctx.enter_context(tc.tile_pool(name="zps", bufs=3, space="PSUM"))

        # constants + x^T + recip row on the gpsimd queue
        wl_sb = const.tile([D, D], bf16)
        nc.gpsimd.dma_start(wl_sb[:], wl_d[:, :])
        wr_sb = const.tile([D, D], bf16)
        nc.gpsimd.dma_start(wr_sb[:], wr_d[:, :])
        b_sb = const.tile([D, 1], f32)
        nc.gpsimd.dma_start(b_sb[:], b_d[:, :])
        rc_tab = const.tile([P, nrank], bf16)
        nc.gpsimd.dma_start(rc_tab[0:1, :], rc_d[:, :])
        xtiles = []
        for g, (c0, c1, wg) in enumerate(groups):
            xt_sb = xt_pool.tile([P, GROUP_BINS * BIN], bf16, tag="xt")
            nc.gpsimd.dma_start(xt_sb[:, :wg], xt_d[:, goff[g]:goff[g] + wg])
            xtiles.append(xt_sb)
        # replicate 1/deg row across all 128 partitions on-chip
        nc.gpsimd.partition_broadcast(rc_tab[:, :], rc_tab[0:1, :])

        # fp8 message + one-hot streams on the sync queue, interleaved so
        # early groups complete first
        gtiles = []
        for g, (c0, c1, wg) in enumerate(groups):
            cg = c1 - c0
            msg_sb = msg8_pool.tile([P, maxc * D], fp8, tag="msg8")
            nc.sync.dma_start(msg_sb[:, :cg * D], msgs_d[:, c0 * D:c1 * D])
            oh_sb = oh_pool.tile([P, maxc * BIN], fp8, tag="oh")
            nc.sync.dma_start(oh_sb[:, :cg * BIN], oh_d[:, c0 * BIN:c1 * BIN])
            gtiles.append((msg_sb, oh_sb))

        for g, (c0, c1, wg) in enumerate(groups):
            cg = c1 - c0
            o0 = goff[g]
            msg_sb, oh_sb = gtiles[g]

            # fp8 -> bf16 message conversion (exact), alternating engines
            m16_sb = msg16_pool.tile([P, maxc * D], bf16, tag="m16")
            conv = nc.vector if g % 2 == 0 else nc.gpsimd
            conv.tensor_scalar_mul(m16_sb[:, :cg * D], msg_sb[:, :cg * D], 1.0)

            mt_ps = mt_psum.tile([P, GROUP_BINS * BIN], f32, space="PSUM")
            for (c, p0, t, win, st, sp) in sched[g]:
                lc = c - c0
                nc.tensor.matmul(
                    out=mt_ps[:, win:win + BIN],
                    lhsT=m16_sb[p0:p0 + t, lc * D:(lc + 1) * D],
                    rhs=oh_sb[p0:p0 + t, lc * BIN:(lc + 1) * BIN],
                    start=bool(st),
                    stop=bool(sp),
                )

            mean_sb = mean_pool.tile([P, GROUP_BINS * BIN], bf16, tag="mt")
            nc.vector.tensor_tensor(
                out=mean_sb[:, :wg],
                in0=mt_ps[:, :wg],
                in1=rc_tab[:, o0:o0 + wg],
                op=mybir.AluOpType.mult,
            )

            z_ps = z_psum.tile([P, GROUP_BINS * BIN], f32, space="PSUM")
            nc.tensor.matmul(out=z_ps[:, :wg], lhsT=wl_sb[:],
                             rhs=mean_sb[:, :wg], start=True, stop=False)
            nc.tensor.matmul(out=z_ps[:, :wg], lhsT=wr_sb[:],
                             rhs=xtiles[g][:, :wg], start=False, stop=True)
            o_sb = out_pool.tile([P, GROUP_BINS * BIN], bf16, tag="o")
            nc.scalar.activation(
                o_sb[:, :wg], z_ps[:, :wg],
                mybir.ActivationFunctionType.Relu, bias=b_sb[:, :1], scale=1.0,
            )
            nc.scalar.dma_start(out_d[:, o0:o0 + wg], o_sb[:, :wg])

    nc.compile()
    return nc


# ---------------------------------------------------------------------------
# Top level
# ---------------------------------------------------------------------------

def _run(inputs, trace=False):
    import ml_dtypes

    from concourse import bass_utils

    x = np.ascontiguousarray(np.asarray(inputs["x"], dtype=np.float32))
    ei = np.asarray(inputs["edge_index"], dtype=np.int64)
    w_l = np.asarray(inputs["W_l"], dtype=np.float32)
    b_l = np.asarray(inputs["b_l"], dtype=np.float32)
    w_r = np.asarray(inputs["W_r"], dtype=np.float32)
    src, dst = ei[0], ei[1]

    meta, in_parts = _prep(x, src, dst, N_CORES)
    nc = _build(meta)

    wl_bf = np.ascontiguousarray(w_l.astype(ml_dtypes.bfloat16))
    wr_bf = np.ascontiguousarray(w_r.astype(ml_dtypes.bfloat16))
    b_col = np.ascontiguousarray(b_l.reshape(D, 1), dtype=np.float32)
    in_maps = []
    for m in range(N_CORES):
        part = in_parts[m]
        in_maps.append({
            "msgs": part["msgs"],
            "oh": part["oh"],
            "xt": part["xt"],
            "recip": part["recip"],
            "wl": wl_bf,
            "wr": wr_bf,
            "bias": b_col,
        })

    results = bass_utils.run_bass_kernel_spmd(
        nc, in_maps, core_ids=list(range(N_CORES)), trace=trace
    )

    n = meta["n"]
    out = np.empty((n, D), dtype=np.float32)
    for m in range(N_CORES):
        out_t = results.results[m]["outT"]  # [128, nrank] feature-major
        nodes_m, pos_m = meta["node_pos"][m]
        out[nodes_m] = out_t[:, pos_m].T.astype(np.float32)
    return out, results


def kernel(**inputs) -> np.ndarray:
    return _run(inputs)[0]


# revision 12
# speedup vs baseline: 4.8576x; 4.8576x over previous
"""GraphSAGE conv layer (PyG SAGEConv, aggr='mean') on 8 Trainium2 NeuronCores.

    out = relu(mean_j(x[src_j]) @ W_l + b_l + x @ W_r)

Sharding: destination nodes are assigned to the 8 cores by a global
degree-balanced bin packing (196 bins x 32 nodes per core); the small 128x128
weights are replicated.

The host does all per-edge indexing; the device is a streaming pipeline with
no gathers:

  - Nodes are dealt into 1568 bins so every bin receives ~equal edge counts;
    bins are then matched across cores by load so one shared per-column
    schedule (bin boundaries, start/stop flags, single NEFF) fits all 8 cores
    with <2% padded edge slots.
  - The per-edge source features stream from HBM as bf16: fp8 stationaries
    serialize the PE weight-load pipeline (~122ns/matmul vs ~27ns for bf16)
    and every on-chip fp8->bf16 conversion path (DVE/GpSimd microcode, ACT
    1 elem/cycle, PE identity matmul) costs more than the DMA it saves.
  - PE: per column, one matmul msgs^T @ onehot (bf16 x fp8) accumulates the
    feature-major per-node segment sum into a PSUM group tile.
  - DVE: multiplies the PSUM sums by 1/deg while casting to bf16; the
    reciprocal table streams as a single 12.5KB row and is replicated
    across partitions on-chip (GpSimd partition_broadcast).
  - PE: weight-stationary bf16 matmuls add W_l^T @ meanT + W_r^T @ xT.
  - ACT: fused bias + ReLU; result stored feature-major as bf16 and the host
    unshuffles/transposes while assembling the full output.

All loads are issued up front (deep prefetch): the fp8 message + one-hot
streams on the sync DMA queue, x^T/constants on the gpsimd queue, stores on
the scalar queue.
"""

import math

import numpy as np

N_CORES = 8
D = 128
P = 128
BIN = 32            # nodes per psum bin (one-hot width)
GROUP_BINS = 16     # bins per psum group -> 512 nodes
NBINS = 196         # bins per core


# ---------------------------------------------------------------------------
# Host-side sharding / stream prep
# ---------------------------------------------------------------------------

def _prep(x, src, dst, n_cores):
    import ml_dtypes

    fp8 = ml_dtypes.float8_e4m3fn
    bf16 = ml_dtypes.bfloat16

    n, d = x.shape
    assert d == D
    nbins_tot = n_cores * NBINS               # 1568
    nrank = NBINS * BIN                       # 6272
    n_groups = math.ceil(NBINS / GROUP_BINS)  # 13
    assert n <= nbins_tot * BIN

    deg = np.bincount(dst, minlength=n).astype(np.int64)
    recip = np.zeros(n, dtype=np.float32)
    nz = deg > 0
    recip[nz] = 1.0 / deg[nz]

    # --- balanced bin packing: snake-deal nodes (sorted by degree) into bins
    order = np.argsort(-deg, kind="stable")
    bin_of_node = np.empty(n, dtype=np.int64)
    slot_of_node = np.empty(n, dtype=np.int64)
    for r in range(math.ceil(n / nbins_tot)):
        chunk = order[r * nbins_tot:(r + 1) * nbins_tot]
        k = len(chunk)
        b = np.arange(k) if r % 2 == 0 else nbins_tot - 1 - np.arange(k)
        bin_of_node[chunk] = b
        slot_of_node[chunk] = r

    # bin load = number of edges whose dst falls in the bin
    load = np.bincount(bin_of_node[dst], minlength=nbins_tot)

    # --- match bins across cores by load: rank i takes the 8 bins of ranks
    # [8i, 8i+8); the shared schedule then pads each core's bin to the max
    # load in its rank group (within a couple of edges of its own count).
    bsort = np.argsort(-load, kind="stable")
    core_of_bin = np.empty(nbins_tot, dtype=np.int64)
    index_of_bin = np.empty(nbins_tot, dtype=np.int64)
    for i in range(NBINS):
        grp = bsort[i * n_cores:(i + 1) * n_cores]
        cores = np.arange(n_cores) if i % 2 == 0 else np.arange(n_cores - 1, -1, -1)
        core_of_bin[grp] = cores
        index_of_bin[grp] = i
    t_need = np.maximum(load[bsort].reshape(NBINS, n_cores)[:, 0], 1)

    C_b = (t_need + P - 1) // P
    F_b = C_b - 1
    # tails quantized to 32 rows: PE matmul operands must sit at partition
    # offsets 0/32/64/96 (and sizes >64 only at 0), so slots are 32-granular.
    t_tail = 32 * ((np.maximum(t_need - F_b * P, 1) + 31) // 32)

    # --- shared column schedule: full 128-row columns per bin plus tails
    # first-fit-decreasing packed into shared 128-row blocks per psum group.
    full_base = np.zeros(NBINS, dtype=np.int64)
    block_col = np.zeros(NBINS, dtype=np.int64)
    block_off = np.zeros(NBINS, dtype=np.int64)
    groups = []          # (c0, c1, wg)
    sched = []           # per group: list of (c, p0, t, win_lo, start, stop)
    ncol = 0
    for g in range(n_groups):
        b0, b1 = g * GROUP_BINS, min((g + 1) * GROUP_BINS, NBINS)
        c0 = ncol
        for b in range(b0, b1):
            full_base[b] = ncol
            ncol += int(F_b[b])
        order2 = sorted(range(b0, b1), key=lambda b: -t_tail[b])
        blocks = []      # (col, remaining rows)
        for b in order2:
            t = int(t_tail[b])
            placed = False
            for bi, (bc, rem) in enumerate(blocks):
                if rem >= t:
                    block_col[b] = bc
                    block_off[b] = P - rem
                    blocks[bi] = (bc, rem - t)
                    placed = True
                    break
            if not placed:
                block_col[b] = ncol
                block_off[b] = 0
                blocks.append((ncol, P - t))
                ncol += 1
        gsched = []
        for b in range(b0, b1):
            for k in range(int(F_b[b])):
                gsched.append((int(full_base[b] + k), 0, P, (b - b0) * BIN,
                               k == 0, False))
            gsched.append((int(block_col[b]), int(block_off[b]), int(t_tail[b]),
                           (b - b0) * BIN, int(F_b[b]) == 0, True))
        groups.append((c0, ncol, (b1 - b0) * BIN))
        sched.append(gsched)
    n_cols = ncol
    maxc = max(c1 - c0 for c0, c1, _ in groups)

    # --- per-core streams
    core_of_node = core_of_bin[bin_of_node]
    pos_of_node = index_of_bin[bin_of_node] * BIN + slot_of_node
    x_bf = x.astype(bf16)
    edge_core = core_of_node[dst]

    in_parts = []
    node_pos = []        # (node ids, local positions) per core, for unshuffle
    for m in range(n_cores):
        sel = edge_core == m
        s = src[sel]
        dn = dst[sel]
        bi = index_of_bin[bin_of_node[dn]]
        order3 = np.argsort(bi, kind="stable")
        s, dn, bi = s[order3], dn[order3], bi[order3]
        cnt = np.bincount(bi, minlength=NBINS)
        bin_start = np.concatenate([[0], np.cumsum(cnt)])
        j = np.arange(len(s)) - bin_start[bi]
        fullslots = F_b[bi] * P
        is_full = j < fullslots
        col = np.where(is_full, full_base[bi] + (j >> 7), block_col[bi])
        p = np.where(is_full, j & 127, block_off[bi] + (j - fullslots))
        lin = col * P + p

        msgs = np.zeros((n_cols * P, D), dtype=bf16)
        msgs[lin] = x_bf[s]
        msgs = msgs.reshape(n_cols, P, D).transpose(1, 0, 2)
        msgs = np.ascontiguousarray(msgs.reshape(P, n_cols * D))

        oh = np.zeros((n_cols * P, BIN), dtype=np.float32)
        oh[lin, slot_of_node[dn]] = 1.0
        oh = oh.reshape(n_cols, P, BIN).transpose(1, 0, 2)
        oh = np.ascontiguousarray(oh.reshape(P, n_cols * BIN)).astype(fp8)

        nodes_m = np.nonzero(core_of_node == m)[0]
        pos_m = pos_of_node[nodes_m]
        xt = np.zeros((P, nrank), dtype=bf16)
        xt[:, pos_m] = x_bf[nodes_m].T
        rc = np.zeros((1, nrank), dtype=bf16)
        rc[0, pos_m] = recip[nodes_m].astype(bf16)

        in_parts.append({
            "msgs": msgs,
            "oh": oh,
            "xt": np.ascontiguousarray(xt),
            "recip": np.ascontiguousarray(rc),
        })
        node_pos.append((nodes_m, pos_m))

    meta = {
        "n": n, "nrank": nrank, "n_cols": n_cols,
        "n_groups": n_groups, "groups": groups, "maxc": maxc,
        "sched": sched, "node_pos": node_pos,
    }
    return meta, in_parts


# ---------------------------------------------------------------------------
# Device kernel builder
# ---------------------------------------------------------------------------

def _build(meta):
    from contextlib import ExitStack

    import concourse.bass as bass  # noqa: F401
    import concourse.mybir as mybir
    import concourse.tile as tile
    from concourse import bacc

    f32 = mybir.dt.float32
    bf16 = mybir.dt.bfloat16
    fp8 = mybir.dt.float8e4
    nrank = meta["nrank"]
    n_cols = meta["n_cols"]
    groups = meta["groups"]
    maxc = meta["maxc"]
    sched = meta["sched"]
    n_groups = meta["n_groups"]

    nc = bacc.Bacc("TRN2", target_bir_lowering=False)
    msgs_d = nc.dram_tensor("msgs", [P, n_cols * D], bf16, kind="ExternalInput")
    oh_d = nc.dram_tensor("oh", [P, n_cols * BIN], fp8, kind="ExternalInput")
    xt_d = nc.dram_tensor("xt", [P, nrank], bf16, kind="ExternalInput")
    rc_d = nc.dram_tensor("recip", [1, nrank], bf16, kind="ExternalInput")
    wl_d = nc.dram_tensor("wl", [D, D], bf16, kind="ExternalInput")
    wr_d = nc.dram_tensor("wr", [D, D], bf16, kind="ExternalInput")
    b_d = nc.dram_tensor("bias", [D, 1], f32, kind="ExternalInput")
    out_d = nc.dram_tensor("outT", [P, nrank], bf16, kind="ExternalOutput")

    goff = [0]
    for _, _, wg in groups:
        goff.append(goff[-1] + wg)

    with ExitStack() as ctx:
        tc = ctx.enter_context(tile.TileContext(nc))
        const = ctx.enter_context(tc.tile_pool(name="const", bufs=1))
        msg_pool = ctx.enter_context(tc.tile_pool(name="msg", bufs=n_groups))
        oh_pool = ctx.enter_context(tc.tile_pool(name="ohp", bufs=n_groups))
        xt_pool = ctx.enter_context(tc.tile_pool(name="xtp", bufs=n_groups))
        mean_pool = ctx.enter_context(tc.tile_pool(name="mean", bufs=2))
        out_pool = ctx.enter_context(tc.tile_pool(name="outp", bufs=2))
        mt_psum = ctx.enter_context(tc.tile_pool(name="mtps", bufs=4, space="PSUM"))
        z_psum = ctx.enter_context(tc.tile_pool(name="zps", bufs=3, space="PSUM"))

        # small constants + recip row first (gpsimd queue)
        wl_sb = const.tile([D, D], bf16)
        nc.gpsimd.dma_start(wl_sb[:], wl_d[:, :])
        wr_sb = const.tile([D, D], bf16)
        nc.gpsimd.dma_start(wr_sb[:], wr_d[:, :])
        b_sb = const.tile([D, 1], f32)
        nc.gpsimd.dma_start(b_sb[:], b_d[:, :])
        rc_tab = const.tile([P, nrank], bf16)
        nc.gpsimd.dma_start(rc_tab[0:1, :], rc_d[:, :])
        # replicate 1/deg row across all 128 partitions on-chip
        nc.gpsimd.partition_broadcast(rc_tab[:, :], rc_tab[0:1, :])

        # deep prefetch: all group loads issued up front, the big message
        # stream alternating between the sync and gpsimd DMA queues
        gtiles = []
        for g, (c0, c1, wg) in enumerate(groups):
            cg = c1 - c0
            eng_a, eng_b = (nc.sync, nc.gpsimd) if g % 2 == 0 else (nc.gpsimd, nc.sync)
            msg_sb = msg_pool.tile([P, maxc * D], bf16, tag="msg")
            eng_a.dma_start(msg_sb[:, :cg * D], msgs_d[:, c0 * D:c1 * D])
            oh_sb = oh_pool.tile([P, maxc * BIN], fp8, tag="oh")
            eng_b.dma_start(oh_sb[:, :cg * BIN], oh_d[:, c0 * BIN:c1 * BIN])
            xt_sb = xt_pool.tile([P, GROUP_BINS * BIN], bf16, tag="xt")
            eng_b.dma_start(xt_sb[:, :wg], xt_d[:, goff[g]:goff[g] + wg])
            gtiles.append((msg_sb, oh_sb, xt_sb))

        for g, (c0, c1, wg) in enumerate(groups):
            cg = c1 - c0
            o0 = goff[g]
            msg_sb, oh_sb, xt_sb = gtiles[g]

            mt_ps = mt_psum.tile([P, GROUP_BINS * BIN], f32, space="PSUM")
            for (c, p0, t, win, st, sp) in sched[g]:
                lc = c - c0
                nc.tensor.matmul(
                    out=mt_ps[:, win:win + BIN],
                    lhsT=msg_sb[p0:p0 + t, lc * D:(lc + 1) * D],
                    rhs=oh_sb[p0:p0 + t, lc * BIN:(lc + 1) * BIN],
                    start=bool(st),
                    stop=bool(sp),
                )

            mean_sb = mean_pool.tile([P, GROUP_BINS * BIN], bf16, tag="mt")
            nc.vector.tensor_tensor(
                out=mean_sb[:, :wg],
                in0=mt_ps[:, :wg],
                in1=rc_tab[:, o0:o0 + wg],
                op=mybir.AluOpType.mult,
            )

            z_ps = z_psum.tile([P, GROUP_BINS * BIN], f32, space="PSUM")
            nc.tensor.matmul(out=z_ps[:, :wg], lhsT=wl_sb[:],
                             rhs=mean_sb[:, :wg], start=True, stop=False)
            nc.tensor.matmul(out=z_ps[:, :wg], lhsT=wr_sb[:],
                             rhs=xt_sb[:, :wg], start=False, stop=True)
            o_sb = out_pool.tile([P, GROUP_BINS * BIN], bf16, tag="o")
            nc.scalar.activation(
                o_sb[:, :wg], z_ps[:, :wg],
                mybir.ActivationFunctionType.Relu, bias=b_sb[:, :1], scale=1.0,
            )
            nc.scalar.dma_start(out_d[:, o0:o0 + wg], o_sb[:, :wg])

    nc.compile()
    return nc


# ---------------------------------------------------------------------------
# Top level
# ---------------------------------------------------------------------------

def _run(inputs, trace=False):
    import ml_dtypes

    from concourse import bass_utils

    x = np.ascontiguousarray(np.asarray(inputs["x"], dtype=np.float32))
    ei = np.asarray(inputs["edge_index"], dtype=np.int64)
    w_l = np.asarray(inputs["W_l"], dtype=np.float32)
    b_l = np.asarray(inputs["b_l"], dtype=np.float32)
    w_r = np.asarray(inputs["W_r"], dtype=np.float32)
    src, dst = ei[0], ei[1]

    meta, in_parts = _prep(x, src, dst, N_CORES)
    nc = _build(meta)

    wl_bf = np.ascontiguousarray(w_l.astype(ml_dtypes.bfloat16))
    wr_bf = np.ascontiguousarray(w_r.astype(ml_dtypes.bfloat16))
    b_col = np.ascontiguousarray(b_l.reshape(D, 1), dtype=np.float32)
    in_maps = []
    for m in range(N_CORES):
        part = in_parts[m]
        in_maps.append({
            "msgs": part["msgs"],
            "oh": part["oh"],
            "xt": part["xt"],
            "recip": part["recip"],
            "wl": wl_bf,
            "wr": wr_bf,
            "bias": b_col,
        })

    results = bass_utils.run_bass_kernel_spmd(
        nc, in_maps, core_ids=list(range(N_CORES)), trace=trace
    )

    n = meta["n"]
    out = np.empty((n, D), dtype=np.float32)
    for m in range(N_CORES):
        out_t = results.results[m]["outT"]  # [128, nrank] feature-major
        nodes_m, pos_m = meta["node_pos"][m]
        out[nodes_m] = out_t[:, pos_m].T.astype(np.float32)
    return out, results


def kernel(**inputs) -> np.ndarray:
    return _run(inputs)[0]


# revision 18
# speedup vs baseline: 5.5580x; 1.1442x over previous
"""GraphSAGE conv layer (PyG SAGEConv, aggr='mean') on 8 Trainium2 NeuronCores.

    out = relu(mean_j(x[src_j]) @ W_l + b_l + x @ W_r)

Sharding: destination nodes are assigned to the 8 cores by a global
degree-balanced bin packing (196 bins x 32 nodes per core); the small 128x128
weights are replicated.

The host does all per-edge indexing; the device is a streaming pipeline with
no gathers:

  - Nodes are dealt into 1568 bins so every bin receives ~equal edge counts;
    bins are then matched across cores by load so one shared per-column
    schedule (bin boundaries, start/stop flags, single NEFF) fits all 8 cores
    with <2% padded edge slots.
  - The per-edge source features stream from HBM as bf16: fp8 stationaries
    serialize the PE weight-load pipeline (~122ns/matmul vs ~27ns for bf16)
    and every on-chip fp8->bf16 conversion path (DVE/GpSimd microcode, ACT
    1 elem/cycle, PE identity matmul) costs more than the DMA it saves.
  - PE: per column, one matmul msgs^T @ onehot (bf16 x fp8) accumulates the
    feature-major per-node segment sum into a PSUM group tile.
  - DVE: multiplies the PSUM sums by 1/deg while casting to bf16; the
    reciprocal table streams as a single 12.5KB row and is replicated
    across partitions on-chip (GpSimd partition_broadcast).
  - PE: weight-stationary bf16 matmuls add W_l^T @ meanT + W_r^T @ xT.
  - ACT: fused bias + ReLU; result stored feature-major as bf16 and the host
    unshuffles/transposes while assembling the full output.

All loads are issued up front (deep prefetch): the fp8 message + one-hot
streams on the sync DMA queue, x^T/constants on the gpsimd queue, stores on
the scalar queue.
"""

import math

import numpy as np

N_CORES = 8
D = 128
P = 128
BIN = 32            # nodes per psum bin (one-hot width)
GROUP_BINS = 16     # bins per psum group -> 512 nodes
NBINS = 196         # bins per core


# ---------------------------------------------------------------------------
# Host-side sharding / stream prep
# ---------------------------------------------------------------------------

def _prep(x, src, dst, n_cores):
    import ml_dtypes

    fp8 = ml_dtypes.float8_e4m3fn
    bf16 = ml_dtypes.bfloat16

    n, d = x.shape
    assert d == D
    nbins_tot = n_cores * NBINS               # 1568
    nrank = NBINS * BIN                       # 6272
    n_groups = math.ceil(NBINS / GROUP_BINS)  # 13
    assert n <= nbins_tot * BIN

    deg = np.bincount(dst, minlength=n).astype(np.int64)
    recip = np.zeros(n, dtype=np.float32)
    nz = deg > 0
    recip[nz] = 1.0 / deg[nz]

    # --- balanced bin packing: snake-deal nodes (sorted by degree) into bins
    order = np.argsort(-deg, kind="stable")
    bin_of_node = np.empty(n, dtype=np.int64)
    slot_of_node = np.empty(n, dtype=np.int64)
    for r in range(math.ceil(n / nbins_tot)):
        chunk = order[r * nbins_tot:(r + 1) * nbins_tot]
        k = len(chunk)
        b = np.arange(k) if r % 2 == 0 else nbins_tot - 1 - np.arange(k)
        bin_of_node[chunk] = b
        slot_of_node[chunk] = r

    # bin load = number of edges whose dst falls in the bin
    load = np.bincount(bin_of_node[dst], minlength=nbins_tot)

    # --- match bins across cores by load: rank i takes the 8 bins of ranks
    # [8i, 8i+8); the shared schedule then pads each core's bin to the max
    # load in its rank group (within a couple of edges of its own count).
    bsort = np.argsort(-load, kind="stable")
    core_of_bin = np.empty(nbins_tot, dtype=np.int64)
    index_of_bin = np.empty(nbins_tot, dtype=np.int64)
    for i in range(NBINS):
        grp = bsort[i * n_cores:(i + 1) * n_cores]
        cores = np.arange(n_cores) if i % 2 == 0 else np.arange(n_cores - 1, -1, -1)
        core_of_bin[grp] = cores
        index_of_bin[grp] = i
    t_need = np.maximum(load[bsort].reshape(NBINS, n_cores)[:, 0], 1)

    C_b = (t_need + P - 1) // P
    F_b = C_b - 1
    # tails quantized to 32 rows: PE matmul operands must sit at partition
    # offsets 0/32/64/96 (and sizes >64 only at 0), so slots are 32-granular.
    t_tail = 32 * ((np.maximum(t_need - F_b * P, 1) + 31) // 32)

    # --- shared column schedule: full 128-row columns per bin plus tails
    # first-fit-decreasing packed into shared 128-row blocks per psum group.
    # EVERY matmul reads the full 128 rows of its column (partial-row
    # stationaries break the PE weight-load wavefront pipelining: ~122ns vs
    # ~27ns per matmul); each matmul instead gets its OWN one-hot slice, so
    # rows belonging to the other bins sharing a tail block are zero there.
    full_base = np.zeros(NBINS, dtype=np.int64)
    block_col = np.zeros(NBINS, dtype=np.int64)
    block_off = np.zeros(NBINS, dtype=np.int64)
    slice_base = np.zeros(NBINS, dtype=np.int64)   # one-hot slice of full k
    slice_tail = np.zeros(NBINS, dtype=np.int64)   # one-hot slice of tail
    groups = []          # (c0, c1, s0, s1, wg)
    sched = []           # per group: list of (col, slice, win_lo, start, stop)
    ncol = 0
    nsl = 0
    for g in range(n_groups):
        b0, b1 = g * GROUP_BINS, min((g + 1) * GROUP_BINS, NBINS)
        c0, s0 = ncol, nsl
        for b in range(b0, b1):
            full_base[b] = ncol
            ncol += int(F_b[b])
        order2 = sorted(range(b0, b1), key=lambda b: -t_tail[b])
        blocks = []      # (col, remaining rows)
        for b in order2:
            t = int(t_tail[b])
            placed = False
            for bi, (bc, rem) in enumerate(blocks):
                if rem >= t:
                    block_col[b] = bc
                    block_off[b] = P - rem
                    blocks[bi] = (bc, rem - t)
                    placed = True
                    break
            if not placed:
                block_col[b] = ncol
                block_off[b] = 0
                blocks.append((ncol, P - t))
                ncol += 1
        gsched = []
        for b in range(b0, b1):
            slice_base[b] = nsl
            for k in range(int(F_b[b])):
                gsched.append((int(full_base[b] + k), nsl, (b - b0) * BIN,
                               k == 0, False))
                nsl += 1
            slice_tail[b] = nsl
            gsched.append((int(block_col[b]), nsl, (b - b0) * BIN,
                           int(F_b[b]) == 0, True))
            nsl += 1
        groups.append((c0, ncol, s0, nsl, (b1 - b0) * BIN))
        sched.append(gsched)
    n_cols = ncol
    n_slices = nsl
    maxc = max(c1 - c0 for c0, c1, _, _, _ in groups)
    maxs = max(s1 - s0 for _, _, s0, s1, _ in groups)

    # --- per-core streams
    core_of_node = core_of_bin[bin_of_node]
    pos_of_node = index_of_bin[bin_of_node] * BIN + slot_of_node
    x_bf = x.astype(bf16)
    edge_core = core_of_node[dst]

    in_parts = []
    node_pos = []        # (node ids, local positions) per core, for unshuffle
    for m in range(n_cores):
        sel = edge_core == m
        s = src[sel]
        dn = dst[sel]
        bi = index_of_bin[bin_of_node[dn]]
        order3 = np.argsort(bi, kind="stable")
        s, dn, bi = s[order3], dn[order3], bi[order3]
        cnt = np.bincount(bi, minlength=NBINS)
        bin_start = np.concatenate([[0], np.cumsum(cnt)])
        j = np.arange(len(s)) - bin_start[bi]
        fullslots = F_b[bi] * P
        is_full = j < fullslots
        col = np.where(is_full, full_base[bi] + (j >> 7), block_col[bi])
        p = np.where(is_full, j & 127, block_off[bi] + (j - fullslots))
        lin = col * P + p

        msgs = np.zeros((n_cols * P, D), dtype=bf16)
        msgs[lin] = x_bf[s]
        msgs = msgs.reshape(n_cols, P, D).transpose(1, 0, 2)
        msgs = np.ascontiguousarray(msgs.reshape(P, n_cols * D))

        # one one-hot slice per matmul (not per column)
        sl = np.where(is_full, slice_base[bi] + (j >> 7), slice_tail[bi])
        lin_oh = sl * P + p
        oh = np.zeros((n_slices * P, BIN), dtype=np.float32)
        oh[lin_oh, slot_of_node[dn]] = 1.0
        oh = oh.reshape(n_slices, P, BIN).transpose(1, 0, 2)
        oh = np.ascontiguousarray(oh.reshape(P, n_slices * BIN)).astype(fp8)

        nodes_m = np.nonzero(core_of_node == m)[0]
        pos_m = pos_of_node[nodes_m]
        xt = np.zeros((P, nrank), dtype=bf16)
        xt[:, pos_m] = x_bf[nodes_m].T
        rc = np.zeros((1, nrank), dtype=bf16)
        rc[0, pos_m] = recip[nodes_m].astype(bf16)

        in_parts.append({
            "msgs": msgs,
            "oh": oh,
            "xt": np.ascontiguousarray(xt),
            "recip": np.ascontiguousarray(rc),
        })
        node_pos.append((nodes_m, pos_m))

    meta = {
        "n": n, "nrank": nrank, "n_cols": n_cols, "n_slices": n_slices,
        "n_groups": n_groups, "groups": groups, "maxc": maxc, "maxs": maxs,
        "sched": sched, "node_pos": node_pos,
    }
    return meta, in_parts


# ---------------------------------------------------------------------------
# Device kernel builder
# ---------------------------------------------------------------------------

def _build(meta):
    from contextlib import ExitStack

    import concourse.bass as bass  # noqa: F401
    import concourse.mybir as mybir
    import concourse.tile as tile
    from concourse import bacc

    f32 = mybir.dt.float32
    bf16 = mybir.dt.bfloat16
    fp8 = mybir.dt.float8e4
    nrank = meta["nrank"]
    n_cols = meta["n_cols"]
    n_slices = meta["n_slices"]
    groups = meta["groups"]
    maxc = meta["maxc"]
    maxs = meta["maxs"]
    sched = meta["sched"]
    n_groups = meta["n_groups"]

    nc = bacc.Bacc("TRN2", target_bir_lowering=False)
    msgs_d = nc.dram_tensor("msgs", [P, n_cols * D], bf16, kind="ExternalInput")
    oh_d = nc.dram_tensor("oh", [P, n_slices * BIN], fp8, kind="ExternalInput")
    xt_d = nc.dram_tensor("xt", [P, nrank], bf16, kind="ExternalInput")
    rc_d = nc.dram_tensor("recip", [1, nrank], bf16, kind="ExternalInput")
    wl_d = nc.dram_tensor("wl", [D, D], bf16, kind="ExternalInput")
    wr_d = nc.dram_tensor("wr", [D, D], bf16, kind="ExternalInput")
    b_d = nc.dram_tensor("bias", [D, 1], f32, kind="ExternalInput")
    out_d = nc.dram_tensor("outT", [P, nrank], bf16, kind="ExternalOutput")

    goff = [0]
    for _, _, _, _, wg in groups:
        goff.append(goff[-1] + wg)

    with ExitStack() as ctx:
        tc = ctx.enter_context(tile.TileContext(nc))
        const = ctx.enter_context(tc.tile_pool(name="const", bufs=1))
        msg_pool = ctx.enter_context(tc.tile_pool(name="msg", bufs=n_groups))
        oh_pool = ctx.enter_context(tc.tile_pool(name="ohp", bufs=n_groups))
        xt_pool = ctx.enter_context(tc.tile_pool(name="xtp", bufs=n_groups))
        mean_pool = ctx.enter_context(tc.tile_pool(name="mean", bufs=2))
        out_pool = ctx.enter_context(tc.tile_pool(name="outp", bufs=2))
        mt_psum = ctx.enter_context(tc.tile_pool(name="mtps", bufs=4, space="PSUM"))
        z_psum = ctx.enter_context(tc.tile_pool(name="zps", bufs=3, space="PSUM"))

        # small constants + recip row first (gpsimd queue)
        wl_sb = const.tile([D, D], bf16)
        nc.gpsimd.dma_start(wl_sb[:], wl_d[:, :])
        wr_sb = const.tile([D, D], bf16)
        nc.gpsimd.dma_start(wr_sb[:], wr_d[:, :])
        b_sb = const.tile([D, 1], f32)
        nc.gpsimd.dma_start(b_sb[:], b_d[:, :])
        rc_tab = const.tile([P, nrank], bf16)
        nc.gpsimd.dma_start(rc_tab[0:1, :], rc_d[:, :])
        # replicate 1/deg row across all 128 partitions on-chip
        nc.gpsimd.partition_broadcast(rc_tab[:, :], rc_tab[0:1, :])

        # deep prefetch: all group loads issued up front, the big message
        # stream alternating between the sync and gpsimd DMA queues
        gtiles = []
        for g, (c0, c1, s0, s1, wg) in enumerate(groups):
            cg = c1 - c0
            sg = s1 - s0
            eng_a, eng_b = (nc.sync, nc.gpsimd) if g % 2 == 0 else (nc.gpsimd, nc.sync)
            msg_sb = msg_pool.tile([P, maxc * D], bf16, tag="msg")
            eng_a.dma_start(msg_sb[:, :cg * D], msgs_d[:, c0 * D:c1 * D])
            oh_sb = oh_pool.tile([P, maxs * BIN], fp8, tag="oh")
            eng_b.dma_start(oh_sb[:, :sg * BIN], oh_d[:, s0 * BIN:s1 * BIN])
            xt_sb = xt_pool.tile([P, GROUP_BINS * BIN], bf16, tag="xt")
            eng_b.dma_start(xt_sb[:, :wg], xt_d[:, goff[g]:goff[g] + wg])
            gtiles.append((msg_sb, oh_sb, xt_sb))

        for g, (c0, c1, s0, s1, wg) in enumerate(groups):
            o0 = goff[g]
            msg_sb, oh_sb, xt_sb = gtiles[g]

            mt_ps = mt_psum.tile([P, GROUP_BINS * BIN], f32, space="PSUM")
            for (c, si, win, st, sp) in sched[g]:
                lc = c - c0
                ls = si - s0
                nc.tensor.matmul(
                    out=mt_ps[:, win:win + BIN],
                    lhsT=msg_sb[:, lc * D:(lc + 1) * D],
                    rhs=oh_sb[:, ls * BIN:(ls + 1) * BIN],
                    start=bool(st),
                    stop=bool(sp),
                )

            mean_sb = mean_pool.tile([P, GROUP_BINS * BIN], bf16, tag="mt")
            nc.vector.tensor_tensor(
                out=mean_sb[:, :wg],
                in0=mt_ps[:, :wg],
                in1=rc_tab[:, o0:o0 + wg],
                op=mybir.AluOpType.mult,
            )

            z_ps = z_psum.tile([P, GROUP_BINS * BIN], f32, space="PSUM")
            nc.tensor.matmul(out=z_ps[:, :wg], lhsT=wl_sb[:],
                             rhs=mean_sb[:, :wg], start=True, stop=False)
            nc.tensor.matmul(out=z_ps[:, :wg], lhsT=wr_sb[:],
                             rhs=xt_sb[:, :wg], start=False, stop=True)
            o_sb = out_pool.tile([P, GROUP_BINS * BIN], bf16, tag="o")
            nc.scalar.activation(
                o_sb[:, :wg], z_ps[:, :wg],
                mybir.ActivationFunctionType.Relu, bias=b_sb[:, :1], scale=1.0,
            )
            nc.scalar.dma_start(out_d[:, o0:o0 + wg], o_sb[:, :wg])

    nc.compile()
    return nc


# ---------------------------------------------------------------------------
# Top level
# ---------------------------------------------------------------------------

def _run(inputs, trace=False):
    import ml_dtypes

    from concourse import bass_utils

    x = np.ascontiguousarray(np.asarray(inputs["x"], dtype=np.float32))
    ei = np.asarray(inputs["edge_index"], dtype=np.int64)
    w_l = np.asarray(inputs["W_l"], dtype=np.float32)
    b_l = np.asarray(inputs["b_l"], dtype=np.float32)
    w_r = np.asarray(inputs["W_r"], dtype=np.float32)
    src, dst = ei[0], ei[1]

    meta, in_parts = _prep(x, src, dst, N_CORES)
    nc = _build(meta)

    wl_bf = np.ascontiguousarray(w_l.astype(ml_dtypes.bfloat16))
    wr_bf = np.ascontiguousarray(w_r.astype(ml_dtypes.bfloat16))
    b_col = np.ascontiguousarray(b_l.reshape(D, 1), dtype=np.float32)
    in_maps = []
    for m in range(N_CORES):
        part = in_parts[m]
        in_maps.append({
            "msgs": part["msgs"],
            "oh": part["oh"],
            "xt": part["xt"],
            "recip": part["recip"],
            "wl": wl_bf,
            "wr": wr_bf,
            "bias": b_col,
        })

    results = bass_utils.run_bass_kernel_spmd(
        nc, in_maps, core_ids=list(range(N_CORES)), trace=trace
    )

    n = meta["n"]
    out = np.empty((n, D), dtype=np.float32)
    for m in range(N_CORES):
        out_t = results.results[m]["outT"]  # [128, nrank] feature-major
        nodes_m, pos_m = meta["node_pos"][m]
        out[nodes_m] = out_t[:, pos_m].T.astype(np.float32)
    return out, results


def kernel(**inputs) -> np.ndarray:
    return _run(inputs)[0]
